# revision 8
# baseline (speedup 1.0000x reference)
"""Multi-head self-attention (B=2, S=2048, E=1024, H=16) on 8 TRN2 cores.

Sharding: batch (2) x head-groups (4) -> 8 cores. Core c handles batch
c//4 and heads [4*(c%4), 4*(c%4)+4). Each core computes QKV projection,
attention, and its partial output projection; the host sums the 4
head-group partials per batch.

Device schedule: the softmax exp is the hard floor -- the ACT engine
runs exp at 1 elem/lane/cycle @1.2GHz, ~139us for this core's 16.8M
score elements -- so the kernel is built as one ACT-clocked attention
pipeline and every projection rides in the PE slack underneath it:

  - scores for a head PAIR land in one [128,1024] PSUM tile (keys on
    partitions, 512 queries per head side by side), so ONE exp
    instruction covers both heads and releases the pair's next score
    matmuls together (they row-pair on PE quadrants, rows 0-63/64-127).
  - AV uses the interleaved ones-column trick ([v|1] stationary) to get
    softmax row-sums for free, and trails the exp stream by >=2 tiles
    through a deep SBUF attn queue so v-projections never stall ACT.
  - QKV projections, v-formation and the output projection are emitted
    through a generator work-queue pumped between attention steps;
    readiness constraints (kT chunk before its score, v tile before its
    AV) force-drain the queue so the PE stream can never deadlock.
  - x/w DMAs are column-chunked so the first score matmul issues ~3us
    in; a dummy exp at t=0 pre-fires the ACT table load.
"""

from collections import deque
from contextlib import ExitStack

import numpy as np
import ml_dtypes

import concourse.bass as bass
import concourse.tile as tile
from concourse import mybir
from concourse.vector_clock import ScopedClock
from concourse.bass_utils import run_bass_kernel_spmd

B, S, E = 2, 2048, 1024
H, DH = 16, 64
NCORES = 8
HL = 4              # heads per core
GF = HL * DH        # 256: local head feature dim
VW = DH + 1         # v block width incl. ones column
BF16 = mybir.dt.bfloat16
F32 = mybir.dt.float32
bf16 = ml_dtypes.bfloat16

P = 128
EK = E // P         # 8 contraction chunks
ST = S // P         # 16 key tiles
SQ = S // 512       # 4 query groups of 512
N_WARM = 80         # HAM warm-up matmuls issued during input DMA
AV_LAG = 2          # exp->AV pipeline distance (score psum double buffer)
PUMP = 2            # projection matmuls pumped per attention step


def _split_excess_waits(nc):
    """Rewrite TPB instructions carrying >1 sem wait.

    This ISA build has a single (wait, update) event slot per 64B TPB
    instruction, but Tile emits instructions with several waits. Excess
    waits move onto same-engine NoOps inserted immediately before the
    instruction — the engine executes its stream in order, so waiting on
    preceding NoOps is equivalent. DMA instructions are exempt (their
    waits live in DGE descriptors, which support several).
    """
    for f in nc.m.functions:
        for bb in f.blocks:
            out = []
            for inst in bb.instructions:
                si = getattr(inst, "sync_info", None)
                waits = list(si.on_wait) if si and si.on_wait else []
                if len(waits) > 1:
                    ups = list(si.on_update) if si.on_update else []
                    assert len(ups) <= 1, f"{inst.name}: multi-update unsupported"
                    for w in waits[:-1]:
                        out.append(
                            mybir.InstNoOp(
                                name=f"I-{nc.next_id()}",
                                engine=inst.engine,
                                sync_info=mybir.SyncInfo(on_wait=[w], on_update=[]),
                                bass_nofuse=True,
                            )
                        )
                    inst.sync_info = mybir.SyncInfo(on_wait=[waits[-1]], on_update=ups)
                out.append(inst)
            bb.instructions[:] = out


class SafeTileContext(tile.TileContext):
    """TileContext whose tail drain splits sem waits across chained SP nops.

    This walrus build rejects >1 sync-wait command on a CTRL instruction;
    the stock tail drain can carry several and fails codegen ("Too many
    sync wait commands"). Semantics are unchanged: SP serially waits on
    every clock sem via nops, then drains and barriers as usual.
    """

    MAX_WAITS_PER_INST = 1

    def _drain_and_barrier(self, tick_clock, wait_clock):
        nc = self.nc
        probe = mybir.InstNoOp(
            name=nc.get_next_instruction_name(), engine=mybir.EngineType.SP
        )
        wait_clock.add_sem_waits(probe, ScopedClock({None: tick_clock.global_clock}))
        waits = list(probe.sync_info.on_wait) if probe.sync_info else []
        k = self.MAX_WAITS_PER_INST
        for i in range(0, len(waits), k):
            nop = nc.sync.nop(nofuse=True, hint="tail_wait")
            nop.ins.sync_info = mybir.SyncInfo(
                on_wait=list(waits[i : i + k]), on_update=[]
            )
        nc.sync.drain()
        nc.all_engine_barrier()
        popped = nc._tile_sem_poison_stack.pop()
        assert popped is self._sem_poison
        nc.clear_and_free_semaphores(list(self.sems.allocated().values()))
        nc.all_engine_barrier()


def _emit(ctx, tc, xt, wqk, wv, wo, y):
    nc = tc.nc
    rc_dram = nc.dram_tensor("rc_dram", [HL, S], F32)

    consts = ctx.enter_context(tc.tile_pool(name="consts", bufs=1))
    attn_pool = ctx.enter_context(tc.tile_pool(name="attn", bufs=22))
    ou_pool = ctx.enter_context(tc.tile_pool(name="ou", bufs=4))
    rs_pool = ctx.enter_context(tc.tile_pool(name="rs", bufs=2))
    rb_pool = ctx.enter_context(tc.tile_pool(name="rb", bufs=4))
    y_pool = ctx.enter_context(tc.tile_pool(name="ystage", bufs=4))
    ps_s_pool = ctx.enter_context(tc.tile_pool(name="ps_s", bufs=2, space="PSUM"))
    ps_av_pool = ctx.enter_context(tc.tile_pool(name="ps_av", bufs=2, space="PSUM"))
    ps_pj_pool = ctx.enter_context(tc.tile_pool(name="ps_pj", bufs=2, space="PSUM"))

    # ---- input DMAs first: SWDGE descriptor generation rides a Q7 core
    # whose ~6us IRAM load gates the first transfer, so nothing may queue
    # ahead of these.  xt is column-chunked so chunk j=0 (the only one the
    # first projections need) lands as early as possible.
    xt_sb = [consts.tile([P, S], BF16, name=f"xt{e}") for e in range(EK)]
    wqk_sb = [consts.tile([P, 2 * GF], BF16, name=f"wqk{e}") for e in range(EK)]
    wv_sb = [consts.tile([P, GF], BF16, name=f"wv{e}") for e in range(EK)]
    wo_sb = [consts.tile([P, E], BF16, name=f"wo{d}") for d in range(2)]

    def dma_xt_chunk(j):
        sl = slice(512 * j, 512 * (j + 1))
        for e in range(EK):
            nc.sync.dma_start(out=xt_sb[e][:, sl], in_=xt[P * e : P * (e + 1), sl])

    dma_xt_chunk(0)
    for e in range(EK):
        nc.sync.dma_start(out=wqk_sb[e], in_=wqk[P * e : P * (e + 1), :])
    dma_xt_chunk(1)
    for e in range(EK):
        nc.sync.dma_start(out=wv_sb[e], in_=wv[P * e : P * (e + 1), :])
    dma_xt_chunk(2)
    dma_xt_chunk(3)
    for d in range(2):
        nc.sync.dma_start(out=wo_sb[d], in_=wo[P * d : P * (d + 1), :])

    # ---- pre-fire the ACT exp table load on a dummy tile and keep the PE
    # HAM window busy until the first real projections can issue.
    warm = consts.tile([P, 64], BF16, name="warm")
    warm_e = consts.tile([P, 64], BF16, name="warm_e")
    nc.vector.memset(warm, 0.0)
    nc.scalar.activation(warm_e, warm, mybir.ActivationFunctionType.Exp, scale=1.0)
    ps_w = ps_pj_pool.tile([64, 64], F32, name="psw", tag="proj")
    for _ in range(N_WARM):
        nc.tensor.matmul(ps_w, warm, warm[:, 0:64], start=True, stop=True)

    v_sb = [consts.tile([P, GF], BF16, name=f"v{it}") for it in range(ST)]
    ones = consts.tile([P, 1], BF16, name="ones")
    nc.vector.memset(ones, 1.0)

    qkT_sb = [consts.tile([P, S], BF16, name=f"qk{m}") for m in range(4)]
    outT_sb = [consts.tile([P, S], BF16, name=f"ot{d}") for d in range(2)]

    # ---- projection / output work-queue generators.  Each yield is one PE
    # matmul's worth of work, tagged with its approximate cost in ns so the
    # pump can meter how much rides in each attention step's PE slack.
    def gen_qk(m, j):
        ps = ps_pj_pool.tile([P, 512], F32, name="psqk", tag="proj")
        for e in range(EK):
            nc.tensor.matmul(
                ps,
                wqk_sb[e][:, P * m : P * (m + 1)],
                xt_sb[e][:, 512 * j : 512 * (j + 1)],
                start=(e == 0),
                stop=(e == EK - 1),
            )
            yield 216
        nc.vector.tensor_copy(qkT_sb[m][:, 512 * j : 512 * (j + 1)], ps)

    def gen_v(it):
        ps = ps_pj_pool.tile([P, GF], F32, name="psv", tag="proj")
        for e in range(EK):
            nc.tensor.matmul(
                ps,
                xt_sb[e][:, P * it : P * (it + 1)],
                wv_sb[e],
                start=(e == 0),
                stop=(e == EK - 1),
            )
            yield 110
        nc.vector.tensor_copy(v_sb[it], ps)

    def gen_out(it):
        y_sb = y_pool.tile([P, E], BF16, name="ysb", tag="y")
        for u in range(2):
            ps_y = ps_pj_pool.tile([P, 512], F32, name="psy", tag="proj")
            for d in range(2):
                nc.tensor.matmul(
                    ps_y,
                    outT_sb[d][:, P * it : P * (it + 1)],
                    wo_sb[d][:, 512 * u : 512 * (u + 1)],
                    start=(d == 0),
                    stop=(d == 1),
                )
                yield 216
            nc.vector.tensor_copy(y_sb[:, 512 * u : 512 * (u + 1)], ps_y)
        nc.sync.dma_start(out=y[P * it : P * (it + 1), :], in_=y_sb)

    gens = {}
    done = {}
    work = deque()

    def add(key, g):
        gens[key] = g
        work.append(key)

    def step_gen(key):
        try:
            return next(gens[key])
        except StopIteration:
            done[key] = True
            return None

    def pump(ns_budget):
        spent = 0
        while spent < ns_budget and work:
            key = work[0]
            if done.get(key):
                work.popleft()
                continue
            c = step_gen(key)
            if c is not None:
                spent += c

    def ensure(key):
        if key not in gens:
            return
        while not done.get(key):
            step_gen(key)

    add(("qk", 2, 1), gen_qk(2, 1))
    add(("qk", 2, 2), gen_qk(2, 2))
    add(("qk", 2, 3), gen_qk(2, 3))
    for it in range(4):
        add(("v", it), gen_v(it))
    add(("qk", 0, 1), gen_qk(0, 1))
    for it in range(4, 8):
        add(("v", it), gen_v(it))
    add(("qk", 0, 2), gen_qk(0, 2))
    for it in range(8, 12):
        add(("v", it), gen_v(it))
    add(("qk", 0, 3), gen_qk(0, 3))
    for it in range(12, 16):
        add(("v", it), gen_v(it))
    for j in range(SQ):
        add(("qk", 3, j), gen_qk(3, j))
        add(("qk", 1, j), gen_qk(1, j))

    # ---- attention pipeline (flat across all (p, g) groups).  EXPs are
    # the ACT-engine clock; AVs trail through `avq` (cross-group) gated on
    # their v tile so a late v never stalls the exp stream.
    def emit_scores(p, g, ik):
        t = ps_s_pool.tile([P, 1024], F32, name="pss", tag="s")
        for h01 in range(2):
            nc.tensor.matmul(
                t[:, 512 * h01 : 512 * (h01 + 1)],
                qkT_sb[2 + p][64 * h01 : 64 * h01 + 64, P * ik : P * (ik + 1)],
                qkT_sb[p][64 * h01 : 64 * h01 + 64, 512 * g : 512 * (g + 1)],
                start=True,
                stop=True,
            )
        return t

    def rowsum_batch(p, g, ats):
        # softmax denominators: 4-way col-tiled ones-matmuls, two exp tiles
        # per 512-cycle round; head h01 accumulates at PSUM partitions
        # {64*h01, 64*h01+32} (even/odd ik partials, summed on DVE later).
        ps_rs = ps_pj_pool.tile([P, 512], F32, name="psrs", tag="proj")
        for r in range(8):
            for dk in range(2):
                for h01 in range(2):
                    pos = 32 * h01 + 64 * dk
                    nc.tensor.matmul(
                        ps_rs[pos : pos + 1, :],
                        ones,
                        ats[2 * r + dk][:, 512 * h01 : 512 * (h01 + 1)],
                        start=(r == 0),
                        stop=(r == 7),
                        tile_position=(0, pos),
                    )
        return ps_rs

    def normalize(p, g, ps_o, ps_rs):
        sl = slice(512 * g, 512 * (g + 1))
        q = ou_pool.tile([97, 512], F32, name="q", tag="ou")
        nc.vector.tensor_copy(q, ps_rs[0:97, :])
        outUs = []
        for h01 in range(2):
            outU = ou_pool.tile([64, 512], F32, name="ou", tag="ou")
            nc.vector.tensor_copy(outU, ps_o[64 * h01 : 64 * (h01 + 1), :])
            outUs.append(outU)
        # DVE tensor_tensor needs both inputs on the same partitions: move
        # the {64,96} partials onto {0,32} first (cross-partition copy is
        # legal), then add lane-aligned.
        qq = rs_pool.tile([33, 512], F32, name="qq", tag="rs")
        nc.vector.tensor_copy(qq, q[64:97, :])
        rs2 = rs_pool.tile([33, 512], F32, name="rs2", tag="rs")
        nc.vector.memset(rs2, 1.0)
        nc.vector.tensor_add(rs2, q[0:33, :], qq)
        rc2 = rs_pool.tile([33, 512], F32, name="rc2", tag="rs")
        nc.vector.reciprocal(rc2, rs2)
        for h01 in range(2):
            h = 2 * p + h01
            nc.sync.dma_start(
                out=rc_dram[h : h + 1, sl], in_=rc2[32 * h01 : 32 * h01 + 1, :]
            )
            rb = rb_pool.tile([64, 512], F32, name="rb", tag="rb")
            nc.gpsimd.dma_start(
                out=rb, in_=rc_dram[h : h + 1, sl].partition_broadcast(64)
            )
            nc.vector.tensor_mul(
                outT_sb[p][64 * h01 : 64 * h01 + 64, sl], outUs[h01], rb
            )

    avq = deque()          # (p, g, ik, at_tile)
    av_state = {}          # (p, g) -> ps_o accumulator bank
    av_count = {}          # (p, g) -> AVs emitted
    group_ats = {}         # (p, g) -> {ik: at}  (kept for the rowsum batch)

    def emit_av(p, g, ik, at):
        if (p, g) not in av_state:
            av_state[(p, g)] = ps_av_pool.tile([P, 512], F32, name="pso", tag="av")
            group_ats[(p, g)] = {}
        ps_o = av_state[(p, g)]
        group_ats[(p, g)][ik] = at
        for h01 in range(2):
            h = 2 * p + h01
            nc.tensor.matmul(
                ps_o[64 * h01 : 64 * (h01 + 1), :],
                v_sb[ik][:, DH * h : DH * (h + 1)],
                at[:, 512 * h01 : 512 * (h01 + 1)],
                start=(ik == 0),
                stop=(ik == ST - 1),
            )
        n = av_count.get((p, g), 0) + 1
        av_count[(p, g)] = n
        if n == ST:
            ps_rs = rowsum_batch(p, g, group_ats.pop((p, g)))
            normalize(p, g, av_state.pop((p, g)), ps_rs)
            if p == 1:
                for it in range(4 * g, 4 * g + 4):
                    add(("out", it), gen_out(it))

    def service_avq(max_avs, force=False):
        n = 0
        while avq and n < max_avs:
            p0, g0, ik0, at0 = avq[0]
            if force:
                ensure(("v", ik0))
            elif not done.get(("v", ik0)):
                break
            avq.popleft()
            emit_av(p0, g0, ik0, at0)
            n += 1
        return n

    for g_ in gen_qk(2, 0):
        pass
    for g_ in gen_qk(0, 0):
        pass

    for p in range(2):
        for g in range(SQ):
            ensure(("qk", p, g))
            for ik in range(ST):
                ensure(("qk", 2 + p, ik // 4))
                t = emit_scores(p, g, ik)
                at = attn_pool.tile([P, 1024], BF16, name="at", tag="at")
                nc.scalar.activation(
                    at, t, mybir.ActivationFunctionType.Exp, scale=float(DH) ** -0.5
                )
                avq.append((p, g, ik, at))
                navs = service_avq(2)
                if len(avq) > 8:
                    service_avq(len(avq) - 8, force=True)
                pump(880 if navs == 0 else 450)

    service_avq(len(avq), force=True)
    while work:
        pump(10000)


def build_nc(split_waits=True):
    nc = bass.Bass(trn_type="TRN2")
    xt = nc.dram_tensor("xt", [E, S], BF16, kind="ExternalInput")
    wqk = nc.dram_tensor("wqk", [E, 2 * GF], BF16, kind="ExternalInput")
    wv = nc.dram_tensor("wv", [E, GF], BF16, kind="ExternalInput")
    wo = nc.dram_tensor("wo", [GF, E], BF16, kind="ExternalInput")
    y = nc.dram_tensor("y", [S, E], BF16, kind="ExternalOutput")
    with SafeTileContext(nc) as tc:
        with ExitStack() as ctx:
            _emit(ctx, tc, xt, wqk, wv, wo, y)
    if split_waits:
        _split_excess_waits(nc)
    return nc


_NC_CACHE = None


def _get_nc():
    global _NC_CACHE
    if _NC_CACHE is None:
        _NC_CACHE = build_nc()
    return _NC_CACHE


def make_in_maps(x, w_qkv, w_out):
    in_maps = []
    for c in range(NCORES):
        b, g = divmod(c, 4)
        q = w_qkv[GF * g : GF * (g + 1)]
        k = w_qkv[1024 + GF * g : 1024 + GF * (g + 1)]
        v = w_qkv[2048 + GF * g : 2048 + GF * (g + 1)]
        in_maps.append(
            {
                "xt": np.ascontiguousarray(np.asarray(x)[b].T).astype(bf16),
                "wqk": np.ascontiguousarray(
                    np.concatenate([q, k], axis=0).T
                ).astype(bf16),
                "wv": np.ascontiguousarray(np.asarray(v).T).astype(bf16),
                "wo": np.ascontiguousarray(
                    np.asarray(w_out)[:, GF * g : GF * (g + 1)].T
                ).astype(bf16),
            }
        )
    return in_maps


def gather_output(results):
    y = np.zeros((B, S, E), np.float32)
    for c in range(NCORES):
        y[c // 4] += results[c]["y"].astype(np.float32)
    return y


def kernel(x, w_qkv, w_out, **run_kwargs):
    nc = _get_nc()
    in_maps = make_in_maps(np.asarray(x), np.asarray(w_qkv), np.asarray(w_out))
    res = run_bass_kernel_spmd(nc, in_maps, core_ids=list(range(NCORES)), **run_kwargs)
    out = gather_output(res.results)
    if run_kwargs:
        kernel.last_results = res
    return out
